# revision 1
# baseline (speedup 1.0000x reference)
"""Trainium2 Bass kernel for nn_Attention_62938450756123.

Reference computation (per batch b):
    oe[s, h] = out_e[s, b, 0:512] + out_e[s, b, 512:1024]      # bidirectional sum
    od[t, h] = out_d[t, b, :]
    S[s, t]  = sum_h oe[s, h] * od[t, h]
    p[s, t]  = exp(S[s, t])                                     # naive, no max-sub
    ctx[t,h] = (sum_s p[s, t] * oe[s, h]) / (sum_s p[s, t])
    out[t, b, h] = ctx[t, h]

Sharding: data-parallel over batch (bs=16) across 8 NeuronCores, 2 batches
per core, no collectives.

Per-core dataflow (all matmuls bf16 on TensorE, f32 PSUM accumulate):
  - load out_e f32, sum halves on VectorE -> oe_nat bf16 [s, h]
  - cast-load out_d -> bf16, DMA-xbar transpose both to h-major layouts
    oeT/odT stored as [128p, 4hc, 2048] with h = hc*128 + p
  - mm1: psum_S[s128, t512] = sum_hc oeT.T @ odT  ; exp on ScalarE -> P bf16
  - mm2: psum_ctx[t128, h512] += P_tile.T @ oe_nat ; psum_den[t128, 1] +=
    P_tile.T @ ones   (same stationary weights)
  - normalize on VectorE (reciprocal + tensor_scalar), DMA out f32
"""

import numpy as np

import concourse.bass as bass
import concourse.tile as tile
from concourse import bacc, mybir
from concourse.bass_utils import run_bass_kernel_spmd

SL, TL, BS, H = 2048, 2048, 16, 512
NCORES = 8
BPC = BS // NCORES  # batches per core

F32 = mybir.dt.float32
BF16 = mybir.dt.bfloat16

NS = SL // 128        # 16 s-tiles
NH = H // 128         # 4 h-chunks
TCHUNK = 512          # t-chunk (one PSUM bank of f32)
NTC = TL // TCHUNK    # 4 t-chunks
TPC = TCHUNK // 128   # 4 t-tiles per chunk


def build():
    nc = bacc.Bacc("TRN2", target_bir_lowering=False, debug=False,
                   num_devices=NCORES)
    out_e = nc.dram_tensor("out_e", [SL, BPC, 2 * H], F32,
                           kind="ExternalInput").ap()
    out_d = nc.dram_tensor("out_d", [TL, BPC, H], F32,
                           kind="ExternalInput").ap()
    out = nc.dram_tensor("out", [TL, BPC, H], F32,
                         kind="ExternalOutput").ap()

    exp = mybir.ActivationFunctionType.Exp

    with tile.TileContext(nc) as tc:
        with (
            tc.tile_pool(name="consts", bufs=1) as consts,
            tc.tile_pool(name="stage_e", bufs=3) as stage_e_pool,
            tc.tile_pool(name="stage_d", bufs=3) as stage_d_pool,
            tc.tile_pool(name="oenat", bufs=2) as oenat_pool,
            tc.tile_pool(name="oet", bufs=2) as oet_pool,
            tc.tile_pool(name="odt", bufs=2) as odt_pool,
            tc.tile_pool(name="pbuf", bufs=2) as p_pool,
            tc.tile_pool(name="osb", bufs=3) as osb_pool,
            tc.tile_pool(name="small", bufs=4) as small_pool,
            tc.tile_pool(name="psS", bufs=3, space="PSUM") as psS_pool,
            tc.tile_pool(name="psC", bufs=2, space="PSUM") as psC_pool,
            tc.tile_pool(name="psD", bufs=2, space="PSUM") as psD_pool,
        ):
            ones = consts.tile([128, 1], BF16, tag="ones")
            nc.vector.memset(ones, 1.0)

            for b in range(BPC):
                oe_nat = oenat_pool.tile([128, NS, H], BF16, tag="oe_nat")
                oeT = oet_pool.tile([128, NH, SL], BF16, tag="oeT")
                odT = odt_pool.tile([128, NH, TL], BF16, tag="odT")

                for i in range(NS):
                    st = stage_e_pool.tile([128, 2 * H], F32, tag="st")
                    nc.sync.dma_start(st, out_e[i * 128:(i + 1) * 128, b, :])
                    nc.vector.tensor_add(oe_nat[:, i, :], st[:, 0:H],
                                         st[:, H:2 * H])
                    nc.sync.dma_start(oeT[:, :, i * 128:(i + 1) * 128],
                                      oe_nat[:, i, :], transpose=True)
                    sd = stage_d_pool.tile([128, H], BF16, tag="sd")
                    # SWDGE cast-load f32 -> bf16
                    nc.gpsimd.dma_start(sd, out_d[i * 128:(i + 1) * 128, b, :])
                    nc.sync.dma_start(odT[:, :, i * 128:(i + 1) * 128],
                                      sd, transpose=True)

                for tci in range(NTC):
                    P = p_pool.tile([128, NS, TCHUNK], BF16, tag="P")
                    for i in range(NS):
                        psS = psS_pool.tile([128, TCHUNK], F32, tag="psS")
                        for c in range(NH):
                            nc.tensor.matmul(
                                psS,
                                oeT[:, c, i * 128:(i + 1) * 128],
                                odT[:, c, tci * TCHUNK:(tci + 1) * TCHUNK],
                                start=(c == 0), stop=(c == NH - 1))
                        nc.scalar.activation(P[:, i, :], psS, exp)
                    for tt in range(TPC):
                        psC = psC_pool.tile([128, H], F32, tag="psC")
                        psD = psD_pool.tile([128, 1], F32, tag="psD")
                        for i in range(NS):
                            lhsT = P[:, i, tt * 128:(tt + 1) * 128]
                            nc.tensor.matmul(psC, lhsT, oe_nat[:, i, :],
                                             start=(i == 0), stop=(i == NS - 1))
                            nc.tensor.matmul(psD, lhsT, ones,
                                             start=(i == 0), stop=(i == NS - 1))
                        rc = small_pool.tile([128, 1], F32, tag="rc")
                        nc.vector.reciprocal(rc, psD)
                        ob = osb_pool.tile([128, H], F32, tag="ob")
                        nc.vector.tensor_scalar(ob, psC, rc, None,
                                                mybir.AluOpType.mult)
                        t0 = tci * TCHUNK + tt * 128
                        nc.sync.dma_start(out[t0:t0 + 128, b, :], ob)

    nc.compile()
    return nc


_nc = None
last_result = None


def kernel(in_e=None, out_e=None, out_d=None, _trace=False, **_unused):
    global _nc, last_result
    if _nc is None:
        _nc = build()
    out_e = np.asarray(out_e, dtype=np.float32)
    out_d = np.asarray(out_d, dtype=np.float32)
    in_maps = []
    for c in range(NCORES):
        sl = slice(c * BPC, (c + 1) * BPC)
        in_maps.append({
            "out_e": np.ascontiguousarray(out_e[:, sl, :]),
            "out_d": np.ascontiguousarray(out_d[:, sl, :]),
        })
    last_result = run_bass_kernel_spmd(_nc, in_maps,
                                       core_ids=list(range(NCORES)),
                                       trace=_trace)
    return np.concatenate(
        [np.asarray(last_result.results[c]["out"]) for c in range(NCORES)],
        axis=1).astype(np.float32)
